# revision 4
# baseline (speedup 1.0000x reference)
import sys

for p in ("/opt/trn_rl_repo",):
    if p not in sys.path:
        sys.path.insert(0, p)

import numpy as np

import concourse.bass as bass
import concourse.bacc as bacc
import concourse.tile as tile
from concourse import mybir
from concourse.bass_utils import run_bass_kernel_spmd

NUM_ROUTED = 256
DIM = 2048
TOPK = 8
ROUTE_SCALE = 2.5
N_CORES = 8
B, S = 4, 4096
TOKENS = B * S              # 16384
TOK_PER_CORE = TOKENS // N_CORES  # 2048
DC = DIM // 128             # 16 contraction chunks
TB = 512                    # token tile (one PSUM bank of f32)
NTB = TOK_PER_CORE // TB    # 4
F32 = mybir.dt.float32
F32R = mybir.dt.float32r

# top-k fixup thresholds (device logits err << TAU; ultra-marginal -> f64)
TAU = 2.5e-3
TAU2 = 5.0e-5

_cache = {}


def _build():
    if "nc" in _cache:
        return _cache["nc"]
    nc = bacc.Bacc()
    # x: dc-major, per-partition runs of TOK_PER_CORE*4B = 8KB
    xt = nc.declare_dram_parameter("xt", [DC, 128, TOK_PER_CORE], F32R, isOutput=False)
    # w: partition-major, per-partition contiguous DC*NUM_ROUTED*4B = 16KB
    wt = nc.declare_dram_parameter("wt", [128, DC, NUM_ROUTED], F32R, isOutput=False)
    out = nc.declare_dram_parameter("scores", [2, 128, TOK_PER_CORE], F32, isOutput=True)

    with tile.TileContext(nc) as tc:
        with (
            tc.tile_pool(name="w", bufs=1) as wpool,
            tc.tile_pool(name="x", bufs=1) as xpool,
            tc.tile_pool(name="o", bufs=2) as opool,
            tc.tile_pool(name="ps", bufs=1, space=bass.MemorySpace.PSUM) as pspool,
        ):
            w_sb = wpool.tile([128, DC, NUM_ROUTED], F32R)
            # split w into two halves so dc=0 matmuls start sooner
            nc.gpsimd.dma_start(w_sb[:, : DC // 2, :], wt[:, : DC // 2, :])
            nc.gpsimd.dma_start(w_sb[:, DC // 2 :, :], wt[:, DC // 2 :, :])
            x_sb = xpool.tile([128, DC, TOK_PER_CORE], F32R)
            for dc in range(DC - 1):
                nc.sync.dma_start(x_sb[:, dc, :], xt[dc])
            # last chunk arrives in tb-sized quarters so the final matmuls
            # (tb-outer at dc=15) start as early as possible
            for tb in range(NTB):
                nc.sync.dma_start(
                    x_sb[:, DC - 1, tb * TB:(tb + 1) * TB],
                    xt[DC - 1, :, tb * TB:(tb + 1) * TB],
                )
            # all 8 PSUM banks as one tile: bank k = ps_all[:, k, :]
            ps_all = pspool.tile([128, 8, TB], F32)
            o_sb = []
            for eh in range(2):
                o_sb.append(opool.tile([128, TOK_PER_CORE], F32, name=f"o{eh}"))
            for dc in range(DC - 1):
                for eh in range(2):
                    for tb in range(NTB):
                        nc.tensor.matmul(
                            ps_all[:, eh * NTB + tb, :],
                            w_sb[:, dc, eh * 128:(eh + 1) * 128],
                            x_sb[:, dc, tb * TB:(tb + 1) * TB],
                            start=(dc == 0),
                            stop=False,
                            skip_group_check=True,
                        )
            # dc=15: tb-outer so each x15 quarter feeds both eh matmuls at
            # once; copy+ship each 1024-token half as soon as its two banks
            # stop, with the two eh halves on parallel engines/rings
            dc = DC - 1
            for tb in range(NTB):
                for eh in range(2):
                    nc.tensor.matmul(
                        ps_all[:, eh * NTB + tb, :],
                        w_sb[:, dc, eh * 128:(eh + 1) * 128],
                        x_sb[:, dc, tb * TB:(tb + 1) * TB],
                        start=False,
                        stop=True,
                        skip_group_check=True,
                    )
                if tb % 2 == 1:
                    lo = (tb - 1) * TB
                    nc.vector.tensor_copy(
                        o_sb[0][:, lo:lo + 2 * TB],
                        ps_all[:, tb - 1:tb + 1, :],
                    )
                    nc.sync.dma_start(
                        out[0, :, lo:lo + 2 * TB], o_sb[0][:, lo:lo + 2 * TB]
                    )
                    nc.scalar.activation(
                        o_sb[1][:, lo:lo + 2 * TB],
                        ps_all[:, NTB + tb - 1:NTB + tb + 1, :],
                        mybir.ActivationFunctionType.Copy,
                    )
                    nc.scalar.dma_start(
                        out[1, :, lo:lo + 2 * TB], o_sb[1][:, lo:lo + 2 * TB]
                    )
    nc.compile()
    _cache["nc"] = nc
    return nc


def _postprocess(logits, x, weight, bias):
    """top-k + normalized sigmoid weights with exact fixup of marginal tokens.

    logits: [TOKENS, NUM_ROUTED] f32 device output (fp32r matmul, small err)
    """
    T = logits.shape[0]
    # top-9 per token, sorted desc
    top9 = np.argpartition(-logits, 9, axis=1)[:, :9]
    v9 = np.take_along_axis(logits, top9, axis=1)
    order9 = np.argsort(-v9, axis=1, kind="stable")
    sv = np.take_along_axis(v9, order9, axis=1)
    gaps = sv[:, :-1] - sv[:, 1:]
    marginal = gaps.min(axis=1) < TAU
    if np.any(np.asarray(bias) != 0):
        # selection is on sigmoid(logits)+bias, which is not logit-monotone
        # for per-expert bias: recompute every token exactly
        marginal[:] = True

    idx9 = np.take_along_axis(top9, order9, axis=1)
    indices = idx9[:, :TOPK].copy()
    vals = sv[:, :TOPK].astype(np.float64)

    scores8 = 1.0 / (1.0 + np.exp(-vals))

    if marginal.any():
        b = np.asarray(bias, np.float64)
        xm = np.asarray(x, np.float32).reshape(T, DIM)[marginal]
        w32 = np.asarray(weight, np.float32)
        lm = xm @ w32.T  # f32 recompute of marginal tokens
        t9 = np.argpartition(-lm, 9, axis=1)[:, :9]
        mv9 = np.take_along_axis(lm, t9, axis=1)
        o9 = np.argsort(-mv9, axis=1, kind="stable")
        msv = np.take_along_axis(mv9, o9, axis=1)
        mgaps = msv[:, :-1] - msv[:, 1:]
        ultra = mgaps.min(axis=1) < TAU2
        lm = lm.astype(np.float64)
        if ultra.any():
            xum = xm[ultra].astype(np.float64)
            lm[ultra] = xum @ w32.T.astype(np.float64)
        # selection in score space: sigmoid(logit) + bias (reference semantics)
        sm = 1.0 / (1.0 + np.exp(-lm))
        selm = sm + b[None, :]
        t8 = np.argpartition(-selm, TOPK, axis=1)[:, :TOPK]
        sel8 = np.take_along_axis(selm, t8, axis=1)
        o8 = np.argsort(-sel8, axis=1, kind="stable")
        indices[marginal] = np.take_along_axis(t8, o8, axis=1)
        scores8[marginal] = np.take_along_axis(
            np.take_along_axis(sm, t8, axis=1), o8, axis=1
        )

    w = scores8 / (scores8.sum(axis=1, keepdims=True) + 1e-20)
    w = (w * ROUTE_SCALE).astype(np.float32)
    return w, indices.astype(np.int32)


def kernel(x, weight, bias, _trace=False, _trace_kwargs=None):
    nc = _build()
    xf = np.asarray(x, np.float32).reshape(TOKENS, DIM)
    w32 = np.asarray(weight, np.float32)
    # wt[p, dc, e] = w[e, dc*128+p]
    wtr = np.ascontiguousarray(w32.T.reshape(DC, 128, NUM_ROUTED).transpose(1, 0, 2))
    in_maps = []
    for i in range(N_CORES):
        xc = np.ascontiguousarray(
            xf[i * TOK_PER_CORE:(i + 1) * TOK_PER_CORE].T
        ).reshape(DC, 128, TOK_PER_CORE)
        in_maps.append({"xt": xc, "wt": wtr})
    res = run_bass_kernel_spmd(
        nc, in_maps, list(range(N_CORES)),
        trace=_trace, **(_trace_kwargs or {})
    )
    parts = [
        res.results[i]["scores"].reshape(NUM_ROUTED, TOK_PER_CORE).T
        for i in range(N_CORES)
    ]
    logits = np.concatenate(parts, axis=0)  # [TOKENS, 256]
    kernel._last_logits = logits
    kernel._last_exec_ns = getattr(res, "exec_time_ns", None)

    w, indices = _postprocess(logits, x, weight, bias)
    return (
        w.reshape(B, S, TOPK),
        indices.reshape(B, S, TOPK),
    )
